# revision 7
# baseline (speedup 1.0000x reference)
"""Trainium2 Bass kernel for nn_AttentionModule.

Computation (per batch row b, input feature i):
    E      = tanh(x @ E_W + E_b)                      # [B, 50]
    s      = einsum('be,iea->bia', E, A_W) + A_b      # [B, 66, 20]
    A      = softmax(s, -1)[..., 1]                   # [B, 66]
    out    = x * A

Key rewrite: softmax(s)[1] = 1 / sum_a exp(s_a - s_1).  We pre-difference the
attention weights on the host (dW = A_W - A_W[:,:,1:2], db likewise), fold the
bias into the matmul via a constant-1 row of E (tanh(0*x + 30) == 1.0), and get

    den[b,i] = sum_a exp(E~ @ W2[:, 20i+a])   (a=1 column is exactly 0 -> exp=1)
    out[b,i] = x[b,i] / den[b,i]

Per-core layout (pure data-parallel over 8 cores, 32768 rows/core):
  - load x in [128, 66] row-blocks; PE-transpose -> xT [66, 512] per 512-macro
  - mm1 (fp32): ET~ [51, 512] = W1~.T @ xT ; ACT tanh(+bias) -> SBUF
  - mm2 (f32r): S [128, 1320] = ET~[:,blk].T @ W2 in 3 PSUM banks
  - ACT exp  PSUM->SBUF (one op per 128-block, strided 3x440 read)
  - GPSIMD folds cols [0:7] += [13:20] in-place; DVE reduces groups of 13
  - DVE reciprocal; GPSIMD multiplies by x; DMA out
"""

import numpy as np

B_TOTAL, INPUT, E_NODE, A_NODE = 262144, 66, 50, 20
N_CORES = 8
B_LOCAL = B_TOTAL // N_CORES          # 32768
NBLK = 4                              # 128-row blocks per macro tile
MACRO = 128 * NBLK                    # 512
NMACRO = B_LOCAL // MACRO             # 64
NIA = INPUT * A_NODE                  # 1320
CHUNK = NIA // 3                      # 440  (mm2 moving-dim chunk, <=512)
FOLD = 7                              # pair-fold width done on GPSIMD
KEEP = A_NODE - FOLD                  # 13   (reduced on DVE)
CONST_ROW_BIAS = 30.0                 # tanh(30) == 1.0 in fp32

# mm2 in float32r (1 cyc/row vs 4 for fp32). Flip to False if accuracy fails.
MM2_F32R = True
USE_GPSIMD = True        # fold + final-mul on GPSIMD (else all on DVE)
EXP_STRIDED = True       # one exp op over [128,3,440] (else 3 per-bank ops)

_CACHE = {}


def _build_bass(n_rows):
    import concourse.bass as bass
    import concourse.bacc as bacc
    import concourse.tile as tile
    from concourse import mybir
    from concourse.masks import make_identity
    from contextlib import ExitStack

    f32 = mybir.dt.float32
    f32r = mybir.dt.float32r
    nmacro = n_rows // MACRO

    nc = bacc.Bacc("TRN2", target_bir_lowering=False, debug=False,
                   num_devices=N_CORES)

    w2_dt = f32r if MM2_F32R else f32
    x_d = nc.dram_tensor("x", [n_rows, INPUT], f32, kind="ExternalInput").ap()
    w1_d = nc.dram_tensor("W1", [INPUT, E_NODE + 1], f32, kind="ExternalInput").ap()
    b1_d = nc.dram_tensor("b1", [E_NODE + 1, 1], f32, kind="ExternalInput").ap()
    w2_d = nc.dram_tensor("W2", [E_NODE + 1, NIA], w2_dt, kind="ExternalInput").ap()
    y_d = nc.dram_tensor("y", [n_rows, INPUT], f32, kind="ExternalOutput").ap()

    x_r = x_d.rearrange("(m p) f -> m p f", p=128)
    y_r = y_d.rearrange("(m p) f -> m p f", p=128)

    with tile.TileContext(nc) as tc, ExitStack() as ctx:
        const = ctx.enter_context(tc.tile_pool(name="const", bufs=1))
        xpool = ctx.enter_context(tc.tile_pool(name="xp", bufs=3))
        xtp = ctx.enter_context(tc.tile_pool(name="xtp", bufs=2))
        etp = ctx.enter_context(tc.tile_pool(name="etp", bufs=2))
        expp = ctx.enter_context(tc.tile_pool(name="expp", bufs=4))
        denp = ctx.enter_context(tc.tile_pool(name="denp", bufs=6))
        outp = ctx.enter_context(tc.tile_pool(name="outp", bufs=3))
        ps_xt = ctx.enter_context(tc.tile_pool(name="ps_xt", bufs=1, space="PSUM"))
        ps_et = ctx.enter_context(tc.tile_pool(name="ps_et", bufs=1, space="PSUM"))
        ps_s = ctx.enter_context(tc.tile_pool(name="ps_s", bufs=2, space="PSUM"))

        w1_sb = const.tile([INPUT, E_NODE + 1], f32)
        nc.sync.dma_start(out=w1_sb, in_=w1_d)
        b1_sb = const.tile([E_NODE + 1, 1], f32)
        nc.sync.dma_start(out=b1_sb, in_=b1_d)
        w2_sb = const.tile([E_NODE + 1, NIA], w2_dt)
        nc.sync.dma_start(out=w2_sb, in_=w2_d)
        ident = const.tile([128, 128], f32)
        make_identity(nc, ident)

        for m in range(nmacro):
            x_sb = xpool.tile([128, NBLK, INPUT], f32)
            nc.sync.dma_start(
                out=x_sb,
                in_=x_r[m * NBLK:(m + 1) * NBLK].rearrange("m p f -> p m f"),
            )

            # x [128,66] blocks -> xT [66, 512] via PE transpose
            xt_ps = ps_xt.tile([INPUT, MACRO], f32)
            for b in range(NBLK):
                nc.tensor.transpose(
                    xt_ps[:, b * 128:(b + 1) * 128], x_sb[:, b, :], ident
                )
            xt_sb = xtp.tile([INPUT, MACRO], f32)
            nc.vector.tensor_copy(out=xt_sb, in_=xt_ps)

            # mm1: ET~ [51, 512] = W1~.T @ xT  (fp32 for accuracy)
            et_ps = ps_et.tile([E_NODE + 1, MACRO], f32)
            nc.tensor.matmul(et_ps, w1_sb, xt_sb, start=True, stop=True)
            et_sb = etp.tile([E_NODE + 1, MACRO], w2_dt)
            nc.scalar.activation(
                et_sb, et_ps, mybir.ActivationFunctionType.Tanh,
                bias=b1_sb, scale=1.0,
            )

            out_sb = outp.tile([128, NBLK, INPUT], f32)
            for b in range(NBLK):
                # mm2: S [128, 1320] in 3 PSUM banks (cols 0/512/1024)
                s_ps = ps_s.tile([128, 3 * 512], f32)
                lhs = et_sb[:, b * 128:(b + 1) * 128]
                for c in range(3):
                    nc.tensor.matmul(
                        s_ps[:, c * 512:c * 512 + CHUNK], lhs,
                        w2_sb[:, c * CHUNK:(c + 1) * CHUNK],
                        start=True, stop=True,
                    )

                # exp PSUM -> SBUF, one ACT op over strided [128, 3, 440]
                exp_sb = expp.tile([128, NIA], f32)
                if EXP_STRIDED:
                    nc.scalar.activation(
                        exp_sb.rearrange("p (c w) -> p c w", w=CHUNK),
                        s_ps.rearrange("p (c w) -> p c w", w=512)[:, :, 0:CHUNK],
                        mybir.ActivationFunctionType.Exp,
                    )
                else:
                    for c in range(3):
                        nc.scalar.activation(
                            exp_sb[:, c * CHUNK:(c + 1) * CHUNK],
                            s_ps[:, c * 512:c * 512 + CHUNK],
                            mybir.ActivationFunctionType.Exp,
                        )

                # grouped sum over a=0..19: GPSIMD folds 7, DVE reduces 13
                g = exp_sb.rearrange("p (g a) -> p g a", a=A_NODE)
                den = denp.tile([128, INPUT], f32)
                rec = denp.tile([128, INPUT], f32)
                if USE_GPSIMD:
                    nc.gpsimd.tensor_tensor(
                        out=g[:, :, 0:FOLD], in0=g[:, :, 0:FOLD],
                        in1=g[:, :, KEEP:A_NODE], op=mybir.AluOpType.add,
                    )
                    nc.vector.tensor_reduce(
                        out=den, in_=g[:, :, 0:KEEP],
                        axis=mybir.AxisListType.X, op=mybir.AluOpType.add,
                    )
                    nc.vector.reciprocal(out=rec, in_=den)
                    nc.gpsimd.tensor_tensor(
                        out=out_sb[:, b, :], in0=x_sb[:, b, :], in1=rec,
                        op=mybir.AluOpType.mult,
                    )
                else:
                    nc.vector.tensor_reduce(
                        out=den, in_=g,
                        axis=mybir.AxisListType.X, op=mybir.AluOpType.add,
                    )
                    nc.vector.reciprocal(out=rec, in_=den)
                    nc.vector.tensor_tensor(
                        out=out_sb[:, b, :], in0=x_sb[:, b, :], in1=rec,
                        op=mybir.AluOpType.mult,
                    )

            nc.sync.dma_start(
                out=y_r[m * NBLK:(m + 1) * NBLK].rearrange("m p f -> p m f"),
                in_=out_sb,
            )

    nc.compile()
    return nc


def _prep_weights(E_W, E_b, A_W, A_b):
    E_W = np.asarray(E_W, dtype=np.float32)
    E_b = np.asarray(E_b, dtype=np.float32)
    A_W = np.asarray(A_W, dtype=np.float32)
    A_b = np.asarray(A_b, dtype=np.float32)
    w1 = np.concatenate([E_W, np.zeros((INPUT, 1), np.float32)], axis=1)
    b1 = np.concatenate([E_b, np.float32([CONST_ROW_BIAS])]).reshape(-1, 1)
    dW = A_W - A_W[:, :, 1:2]                       # [66, 50, 20]
    db = A_b - A_b[:, 1:2]                          # [66, 20]
    w2 = np.concatenate(
        [dW.transpose(1, 0, 2).reshape(E_NODE, NIA),
         db.reshape(1, NIA)], axis=0,
    ).astype(np.float32)                            # [51, 1320]
    return np.ascontiguousarray(w1), np.ascontiguousarray(b1), \
        np.ascontiguousarray(w2)


def _run(x, E_W, E_b, A_W, A_b, trace=False):
    from concourse.bass_utils import run_bass_kernel_spmd

    x = np.ascontiguousarray(np.asarray(x, dtype=np.float32))
    n_rows_local = x.shape[0] // N_CORES
    key = ("nc", n_rows_local)
    if key not in _CACHE:
        _CACHE[key] = _build_bass(n_rows_local)
    nc = _CACHE[key]

    w1, b1, w2 = _prep_weights(E_W, E_b, A_W, A_b)
    in_maps = [
        {"x": x[i * n_rows_local:(i + 1) * n_rows_local],
         "W1": w1, "b1": b1, "W2": w2}
        for i in range(N_CORES)
    ]
    res = run_bass_kernel_spmd(nc, in_maps, list(range(N_CORES)), trace=trace)
    out = np.concatenate([res.results[i]["y"] for i in range(N_CORES)], axis=0)
    return out, res


def kernel(x, E_W, E_b, A_W, A_b):
    out, _ = _run(x, E_W, E_b, A_W, A_b, trace=False)
    return out


# revision 14
# speedup vs baseline: 27.1712x; 27.1712x over previous
"""Trainium2 Bass kernel for nn_AttentionModule.

Computation (per batch row b, input feature i):
    E      = tanh(x @ E_W + E_b)                      # [B, 50]
    s      = einsum('be,iea->bia', E, A_W) + A_b      # [B, 66, 20]
    A      = softmax(s, -1)[..., 1]                   # [B, 66]
    out    = x * A

Key rewrite: softmax(s)[1] = 1 / sum_a exp(s_a - s_1).  We pre-difference the
attention weights on the host (dW = A_W - A_W[:,:,1:2], db likewise), fold the
bias into the matmul via a constant-1 row of E (tanh(0*x + 30) == 1.0), and get

    den[b,i] = sum_a exp(E~ @ W2[:, 20i+a])   (a=1 column is exactly 0 -> exp=1)
    out[b,i] = x[b,i] / den[b,i]

Per-core layout (pure data-parallel over 8 cores, 32768 rows/core):
  - load x in [128, 66] row-blocks; PE-transpose -> xT [66, 512] per 512-macro
  - mm1 (fp32): ET~ [51, 512] = W1~.T @ xT ; ACT tanh(+bias) -> SBUF
  - mm2 (f32r): S [128, 1320] = ET~[:,blk].T @ W2 in 3 PSUM banks
  - ACT exp  PSUM->SBUF (one op per 128-block, strided 3x440 read)
  - GPSIMD folds cols [0:7] += [13:20] in-place; DVE reduces groups of 13
  - DVE reciprocal; GPSIMD multiplies by x; DMA out
"""

import numpy as np

B_TOTAL, INPUT, E_NODE, A_NODE = 262144, 66, 50, 20
N_CORES = 8
B_LOCAL = B_TOTAL // N_CORES          # 32768
NBLK = 4                              # 128-row blocks per macro tile
MACRO = 128 * NBLK                    # 512
NMACRO = B_LOCAL // MACRO             # 64
A_RED = A_NODE - 1                    # 19  (a=1 column dropped: exp==1 -> +1)
NIA = INPUT * A_RED                   # 1254
CHUNK = NIA // 3                      # 418  (mm2 moving-dim chunk, <=512)
FOLD = 7                              # pair-fold width done on GPSIMD
KEEP = A_RED - FOLD                   # 12   (reduced on DVE)
CONST_ROW_BIAS = 30.0                 # tanh(30) == 1.0 in fp32

# mm2 in float32r (1 cyc/row vs 4 for fp32). Flip to False if accuracy fails.
MM2_F32R = True
USE_GPSIMD = True        # fold + final-mul on GPSIMD (else all on DVE)
EXP_STRIDED = True       # one exp op over [128,3,440] (else 3 per-bank ops)

_CACHE = {}


def _build_bass(n_rows, repeat=1):
    import concourse.bass as bass
    import concourse.bacc as bacc
    import concourse.tile as tile
    from concourse import mybir
    from concourse.masks import make_identity
    from contextlib import ExitStack

    f32 = mybir.dt.float32
    f32r = mybir.dt.float32r
    nmacro = n_rows // MACRO

    nc = bacc.Bacc("TRN2", target_bir_lowering=False, debug=False,
                   num_devices=N_CORES)

    w2_dt = f32r if MM2_F32R else f32
    x_d = nc.dram_tensor("x", [n_rows, INPUT], f32, kind="ExternalInput").ap()
    w1_d = nc.dram_tensor("W1", [INPUT, E_NODE + 1], f32, kind="ExternalInput").ap()
    b1_d = nc.dram_tensor("b1", [E_NODE + 1, 1], f32, kind="ExternalInput").ap()
    w2_d = nc.dram_tensor("W2", [E_NODE + 1, NIA], w2_dt, kind="ExternalInput").ap()
    y_d = nc.dram_tensor("y", [n_rows, INPUT], f32, kind="ExternalOutput").ap()

    x_r = x_d.rearrange("(m p) f -> m p f", p=128)
    y_r = y_d.rearrange("(m p) f -> m p f", p=128)

    with tile.TileContext(nc) as tc, ExitStack() as ctx:
        const = ctx.enter_context(tc.tile_pool(name="const", bufs=1))
        xpool = ctx.enter_context(tc.tile_pool(name="xp", bufs=3))
        xtp = ctx.enter_context(tc.tile_pool(name="xtp", bufs=2))
        etp = ctx.enter_context(tc.tile_pool(name="etp", bufs=2))
        expp = ctx.enter_context(tc.tile_pool(name="expp", bufs=6))
        denp = ctx.enter_context(tc.tile_pool(name="denp", bufs=8))
        outp = ctx.enter_context(tc.tile_pool(name="outp", bufs=3))
        ps_xt = ctx.enter_context(tc.tile_pool(name="ps_xt", bufs=1, space="PSUM"))
        ps_et = ctx.enter_context(tc.tile_pool(name="ps_et", bufs=1, space="PSUM"))
        ps_s = ctx.enter_context(tc.tile_pool(name="ps_s", bufs=2, space="PSUM"))

        w1_sb = const.tile([INPUT, E_NODE + 1], f32)
        nc.sync.dma_start(out=w1_sb, in_=w1_d)
        b1_sb = const.tile([E_NODE + 1, 1], f32)
        nc.sync.dma_start(out=b1_sb, in_=b1_d)
        w2_sb = const.tile([E_NODE + 1, NIA], w2_dt)
        nc.sync.dma_start(out=w2_sb, in_=w2_d)
        ident = const.tile([128, 128], f32)
        make_identity(nc, ident)

        def emit_head(m):
            """x DMA -> PE transpose -> DVE copy -> mm1 -> tanh for macro m."""
            x_sb = xpool.tile([128, NBLK, INPUT], f32)
            nc.sync.dma_start(
                out=x_sb,
                in_=x_r[m * NBLK:(m + 1) * NBLK].rearrange("m p f -> p m f"),
            )
            xt_ps = ps_xt.tile([INPUT, MACRO], f32)
            for b in range(NBLK):
                nc.tensor.transpose(
                    xt_ps[:, b * 128:(b + 1) * 128], x_sb[:, b, :], ident
                )
            xt_sb = xtp.tile([INPUT, MACRO], f32)
            nc.vector.tensor_copy(out=xt_sb, in_=xt_ps)
            et_ps = ps_et.tile([E_NODE + 1, MACRO], f32)
            nc.tensor.matmul(et_ps, w1_sb, xt_sb, start=True, stop=True)
            et_sb = etp.tile([E_NODE + 1, MACRO], w2_dt)
            nc.scalar.activation(
                et_sb, et_ps, mybir.ActivationFunctionType.Tanh,
                bias=b1_sb, scale=1.0,
            )
            return x_sb, et_sb

        iters = [m for _ in range(repeat) for m in range(nmacro)]
        heads = {0: emit_head(iters[0])}
        for it in range(len(iters)):
            m = iters[it]
            if it + 1 < len(iters):
                heads[it + 1] = emit_head(iters[it + 1])
            x_sb, et_sb = heads.pop(it)

            out_sb = outp.tile([128, NBLK, INPUT], f32)
            for b in range(NBLK):
                # mm2: S [128, 1320] in 3 PSUM banks (cols 0/512/1024)
                s_ps = ps_s.tile([128, 3 * 512], f32)
                lhs = et_sb[:, b * 128:(b + 1) * 128]
                for c in range(3):
                    nc.tensor.matmul(
                        s_ps[:, c * 512:c * 512 + CHUNK], lhs,
                        w2_sb[:, c * CHUNK:(c + 1) * CHUNK],
                        start=True, stop=True,
                    )

                # exp PSUM -> SBUF, one ACT op over strided [128, 3, 440]
                exp_sb = expp.tile([128, NIA], f32)
                if EXP_STRIDED:
                    nc.scalar.activation(
                        exp_sb.rearrange("p (c w) -> p c w", w=CHUNK),
                        s_ps.rearrange("p (c w) -> p c w", w=512)[:, :, 0:CHUNK],
                        mybir.ActivationFunctionType.Exp,
                    )
                else:
                    for c in range(3):
                        nc.scalar.activation(
                            exp_sb[:, c * CHUNK:(c + 1) * CHUNK],
                            s_ps[:, c * 512:c * 512 + CHUNK],
                            mybir.ActivationFunctionType.Exp,
                        )

                # den = 1 + grouped sum over the 19 kept columns:
                # GPSIMD folds 7 in-place, DVE reduces 12, GPSIMD adds 1
                g = exp_sb.rearrange("p (g a) -> p g a", a=A_RED)
                den = denp.tile([128, INPUT], f32)
                rec = denp.tile([128, INPUT], f32)
                if USE_GPSIMD:
                    nc.gpsimd.tensor_tensor(
                        out=g[:, :, 0:FOLD], in0=g[:, :, 0:FOLD],
                        in1=g[:, :, KEEP:A_RED], op=mybir.AluOpType.add,
                    )
                    nc.vector.tensor_reduce(
                        out=den, in_=g[:, :, 0:KEEP],
                        axis=mybir.AxisListType.X, op=mybir.AluOpType.add,
                    )
                    nc.gpsimd.tensor_scalar_add(out=den, in0=den, scalar1=1.0)
                    nc.vector.reciprocal(out=rec, in_=den)
                    nc.gpsimd.tensor_tensor(
                        out=out_sb[:, b, :], in0=x_sb[:, b, :], in1=rec,
                        op=mybir.AluOpType.mult,
                    )
                else:
                    nc.vector.tensor_reduce(
                        out=den, in_=g,
                        axis=mybir.AxisListType.X, op=mybir.AluOpType.add,
                    )
                    nc.vector.tensor_scalar_add(out=den, in0=den, scalar1=1.0)
                    nc.vector.reciprocal(out=rec, in_=den)
                    nc.vector.tensor_tensor(
                        out=out_sb[:, b, :], in0=x_sb[:, b, :], in1=rec,
                        op=mybir.AluOpType.mult,
                    )

            nc.sync.dma_start(
                out=y_r[m * NBLK:(m + 1) * NBLK].rearrange("m p f -> p m f"),
                in_=out_sb,
            )

    nc.compile()
    return nc


def _prep_weights(E_W, E_b, A_W, A_b):
    E_W = np.asarray(E_W, dtype=np.float32)
    E_b = np.asarray(E_b, dtype=np.float32)
    A_W = np.asarray(A_W, dtype=np.float32)
    A_b = np.asarray(A_b, dtype=np.float32)
    w1 = np.concatenate([E_W, np.zeros((INPUT, 1), np.float32)], axis=1)
    b1 = np.concatenate([E_b, np.float32([CONST_ROW_BIAS])]).reshape(-1, 1)
    dW = np.delete(A_W - A_W[:, :, 1:2], 1, axis=2)  # [66, 50, 19]
    db = np.delete(A_b - A_b[:, 1:2], 1, axis=1)     # [66, 19]
    w2 = np.concatenate(
        [dW.transpose(1, 0, 2).reshape(E_NODE, NIA),
         db.reshape(1, NIA)], axis=0,
    ).astype(np.float32)                             # [51, 1254]
    return np.ascontiguousarray(w1), np.ascontiguousarray(b1), \
        np.ascontiguousarray(w2)


def _run(x, E_W, E_b, A_W, A_b, trace=False):
    from concourse.bass_utils import run_bass_kernel_spmd

    x = np.ascontiguousarray(np.asarray(x, dtype=np.float32))
    n_rows_local = x.shape[0] // N_CORES
    key = ("nc", n_rows_local)
    if key not in _CACHE:
        _CACHE[key] = _build_bass(n_rows_local)
    nc = _CACHE[key]

    w1, b1, w2 = _prep_weights(E_W, E_b, A_W, A_b)
    in_maps = [
        {"x": x[i * n_rows_local:(i + 1) * n_rows_local],
         "W1": w1, "b1": b1, "W2": w2}
        for i in range(N_CORES)
    ]
    res = run_bass_kernel_spmd(nc, in_maps, list(range(N_CORES)), trace=trace)
    out = np.concatenate([res.results[i]["y"] for i in range(N_CORES)], axis=0)
    return out, res


def kernel(x, E_W, E_b, A_W, A_b):
    out, _ = _run(x, E_W, E_b, A_W, A_b, trace=False)
    return out


# revision 43
# speedup vs baseline: 29.4315x; 1.0832x over previous
"""Trainium2 Bass kernel for nn_AttentionModule.

Computation (per batch row b, input feature i):
    E      = tanh(x @ E_W + E_b)                      # [B, 50]
    s      = einsum('be,iea->bia', E, A_W) + A_b      # [B, 66, 20]
    A      = softmax(s, -1)[..., 1]                   # [B, 66]
    out    = x * A

Key rewrite: softmax(s)[1] = 1 / sum_a exp(s_a - s_1).  We pre-difference the
attention weights on the host (dW = A_W - A_W[:,:,1:2], db likewise), fold the
bias into the matmul via a constant-1 row of E (tanh(0*x + 30) == 1.0), and get

    den[b,i] = sum_a exp(E~ @ W2[:, 20i+a])   (a=1 column is exactly 0 -> exp=1)
    out[b,i] = x[b,i] / den[b,i]

Per-core layout (pure data-parallel over 8 cores, 32768 rows/core):
  - batched DMA loads x for 2 macros (8x128 rows) at a time
  - per 512-row macro: PE-transpose x blocks -> xT [66, 512] (PSUM),
    DVE copy to SBUF, mm1 (fp32): ET~ [51, 512] = W1~.T @ xT,
    ACT tanh(+bias, const-1 row via tanh(30)) -> SBUF (f32r)
  - per 128-row block: mm2 (f32r, 1 cyc/row): S [128, 1254] =
    ET~[:, blk].T @ W2 into 3 PSUM banks; ACT exp in ONE op
    (strided [128, 3, 418] PSUM read -> contiguous SBUF)
  - DVE tensor_reduce over groups of 19 -> den; +1 (dropped a=1 column);
    DVE reciprocal; GPSIMD multiplies by x; batched DMA out

Engine-placement choices are HW-A/B-tested (no NTFF profiling available in
this container): GPSIMD handles ONLY the final contiguous multiply (its
strided/2-input throughput is far worse than documented); the grouped
reduce lives on DVE; per-op overheads dominate, so ops are minimized and
the exp is a single strided ACTIVATE per block.
"""

import numpy as np

B_TOTAL, INPUT, E_NODE, A_NODE = 262144, 66, 50, 20
N_CORES = 8
B_LOCAL = B_TOTAL // N_CORES          # 32768
NBLK = 4                              # 128-row blocks per macro tile
MACRO = 128 * NBLK                    # 512
NMACRO = B_LOCAL // MACRO             # 64
DROP_A1 = True                        # drop a=1 column (exp==1; add 1 later)
FOLD = 0                              # pair-fold width done on GPSIMD


def _dims():
    a_red = A_NODE - 1 if DROP_A1 else A_NODE
    nia = INPUT * a_red
    return a_red, nia, nia // 3
CONST_ROW_BIAS = 30.0                 # tanh(30) == 1.0 in fp32

# mm2 in float32r (1 cyc/row vs 4 for fp32). Flip to False if accuracy fails.
MM2_F32R = True
EXP_STRIDED = True       # one exp op over [128,3,418] (else 3 per-bank ops)
ADD1_ON = "dve"          # engine for the +1 (a=1 softmax column)
MUL_ON = "pool"          # engine for the final x*rec multiply
DIV_TT = False           # out = x/den via one DVE divide (skip recip+mul)
BUFS_UP = False          # deeper SBUF tile pools
TAIL_GROUP = 1           # blocks per reduce/add1/recip/mul op group (1|2|4)
XCOPY_ON = "dve"         # engine for the xT PSUM->SBUF copy
REDUCE_OP = "reduce"     # "reduce" (tensor_reduce) | "pool" (pool_avg+fixup)
POOL_DIV = False         # out = x/den via gpsimd divide (skips DVE recip)
DMA_MACROS = 2           # macros per x-load/y-store DMA
ABLATE = ""              # timing ablations (break numerics):
                         #   "den" | "mm2" | "out" | "expdve"

_CACHE = {}


def _build_bass(n_rows, repeat=1):
    import concourse.bass as bass
    import concourse.bacc as bacc
    import concourse.tile as tile
    from concourse import mybir
    from concourse.masks import make_identity
    from contextlib import ExitStack

    f32 = mybir.dt.float32
    f32r = mybir.dt.float32r
    nmacro = n_rows // MACRO
    A_RED, NIA, CHUNK = _dims()

    nc = bacc.Bacc("TRN2", target_bir_lowering=False, debug=False,
                   num_devices=N_CORES)

    w2_dt = f32r if MM2_F32R else f32
    x_d = nc.dram_tensor("x", [n_rows, INPUT], f32, kind="ExternalInput").ap()
    w1_d = nc.dram_tensor("W1", [INPUT, E_NODE + 1], f32, kind="ExternalInput").ap()
    b1_d = nc.dram_tensor("b1", [E_NODE + 1, 1], f32, kind="ExternalInput").ap()
    w2_d = nc.dram_tensor("W2", [E_NODE + 1, NIA], w2_dt, kind="ExternalInput").ap()
    y_d = nc.dram_tensor("y", [n_rows, INPUT], f32, kind="ExternalOutput").ap()

    x_r = x_d.rearrange("(m p) f -> m p f", p=128)
    y_r = y_d.rearrange("(m p) f -> m p f", p=128)

    with tile.TileContext(nc) as tc, ExitStack() as ctx:
        up = 1 + bool(BUFS_UP)
        const = ctx.enter_context(tc.tile_pool(name="const", bufs=1))
        xpool = ctx.enter_context(tc.tile_pool(name="xp", bufs=3 * up))
        xtp = ctx.enter_context(tc.tile_pool(name="xtp", bufs=2 * up))
        etp = ctx.enter_context(tc.tile_pool(name="etp", bufs=2 * up))
        expp = ctx.enter_context(tc.tile_pool(name="expp", bufs=6 * up))
        denp = ctx.enter_context(tc.tile_pool(name="denp", bufs=8 * up))
        outp = ctx.enter_context(tc.tile_pool(name="outp", bufs=3 * up))
        ps_xt = ctx.enter_context(tc.tile_pool(name="ps_xt", bufs=1, space="PSUM"))
        ps_et = ctx.enter_context(tc.tile_pool(name="ps_et", bufs=1, space="PSUM"))
        ps_s = ctx.enter_context(tc.tile_pool(name="ps_s", bufs=2, space="PSUM"))

        w1_sb = const.tile([INPUT, E_NODE + 1], f32)
        nc.sync.dma_start(out=w1_sb, in_=w1_d)
        b1_sb = const.tile([E_NODE + 1, 1], f32)
        nc.sync.dma_start(out=b1_sb, in_=b1_d)
        w2_sb = const.tile([E_NODE + 1, NIA], w2_dt)
        nc.sync.dma_start(out=w2_sb, in_=w2_d)
        ident = const.tile([128, 128], f32)
        make_identity(nc, ident)

        DM = DMA_MACROS
        assert nmacro % DM == 0
        iters = [m for _ in range(repeat) for m in range(nmacro)]
        xgs = {}

        def emit_load(git):
            """One batched x DMA covering DM consecutive macros."""
            m0 = iters[git * DM]
            xg = xpool.tile([128, DM * NBLK, INPUT], f32)
            nc.sync.dma_start(
                out=xg,
                in_=x_r[m0 * NBLK:m0 * NBLK + DM * NBLK]
                .rearrange("m p f -> p m f"),
            )
            return xg

        def emit_head(it):
            """PE transpose -> copy -> mm1 -> tanh for iteration it."""
            git, off = it // DM, it % DM
            if git not in xgs:
                xgs[git] = emit_load(git)
            x_sb = xgs[git][:, off * NBLK:(off + 1) * NBLK, :]
            xt_ps = ps_xt.tile([INPUT, MACRO], f32)
            for b in range(NBLK):
                nc.tensor.transpose(
                    xt_ps[:, b * 128:(b + 1) * 128], x_sb[:, b, :], ident
                )
            xt_sb = xtp.tile([INPUT, MACRO], f32)
            if XCOPY_ON == "act":
                nc.scalar.copy(out=xt_sb, in_=xt_ps)
            else:
                nc.vector.tensor_copy(out=xt_sb, in_=xt_ps)
            et_ps = ps_et.tile([E_NODE + 1, MACRO], f32)
            nc.tensor.matmul(et_ps, w1_sb, xt_sb, start=True, stop=True)
            et_sb = etp.tile([E_NODE + 1, MACRO], w2_dt)
            nc.scalar.activation(
                et_sb, et_ps, mybir.ActivationFunctionType.Tanh,
                bias=b1_sb, scale=1.0,
            )
            return x_sb, et_sb

        ogs = {}
        heads = {0: emit_head(0)}
        for it in range(len(iters)):
            m = iters[it]
            git, off = it // DM, it % DM
            if it + 1 < len(iters):
                heads[it + 1] = emit_head(it + 1)
            x_sb, et_sb = heads.pop(it)

            if git not in ogs:
                ogs[git] = outp.tile([128, DM * NBLK, INPUT], f32, name="og")
            out_sb = ogs[git][:, off * NBLK:(off + 1) * NBLK, :]
            TG = TAIL_GROUP
            exp_g = None
            for b in range(NBLK):
                bg = b % TG
                # mm2: S [128, NIA] in 3 PSUM banks (cols 0/512/1024)
                s_ps = ps_s.tile([128, 3 * 512], f32)
                lhs = et_sb[:, b * 128:(b + 1) * 128]
                nchunks = 1 if ABLATE == "mm2" else 3
                for c in range(nchunks):
                    nc.tensor.matmul(
                        s_ps[:, c * 512:c * 512 + CHUNK], lhs,
                        w2_sb[:, c * CHUNK:(c + 1) * CHUNK],
                        start=True, stop=True,
                    )

                # exp PSUM -> SBUF, one ACT op over strided [128, 3, CHUNK]
                if bg == 0:
                    exp_g = expp.tile([128, TG * NIA], f32, name="exp_g")
                exp_sb = exp_g[:, bg * NIA:(bg + 1) * NIA]
                if ABLATE == "mm2":
                    a0 = s_ps[:, 0:CHUNK]
                    src = bass.AP(tensor=a0.tensor, offset=a0.offset,
                                  ap=[list(a0.ap[0]), [0, 3], list(a0.ap[1])])
                    nc.scalar.activation(
                        exp_sb.rearrange("p (c w) -> p c w", w=CHUNK),
                        src, mybir.ActivationFunctionType.Exp,
                    )
                elif ABLATE == "expdve":
                    nc.vector.tensor_copy(
                        out=exp_sb.rearrange("p (c w) -> p c w", w=CHUNK),
                        in_=s_ps.rearrange("p (c w) -> p c w",
                                           w=512)[:, :, 0:CHUNK],
                    )
                elif EXP_STRIDED:
                    nc.scalar.activation(
                        exp_sb.rearrange("p (c w) -> p c w", w=CHUNK),
                        s_ps.rearrange("p (c w) -> p c w", w=512)[:, :, 0:CHUNK],
                        mybir.ActivationFunctionType.Exp,
                    )
                else:
                    for c in range(3):
                        nc.scalar.activation(
                            exp_sb[:, c * CHUNK:(c + 1) * CHUNK],
                            s_ps[:, c * 512:c * 512 + CHUNK],
                            mybir.ActivationFunctionType.Exp,
                        )

                if ABLATE == "den":
                    # timing ablation: skip the whole den chain; read exp_sb
                    # so DCE can't eat the exp/mm2 producers
                    nc.gpsimd.tensor_scalar_mul(
                        out=out_sb[:, b, :], in0=exp_sb[:, 0:INPUT],
                        scalar1=1.0)
                    continue
                if bg != TG - 1:
                    continue

                # den = 1 + grouped sum over the 19 kept columns, for the
                # whole TG-block group in single wide ops
                b0 = b - bg
                g = exp_g.rearrange("p (g a) -> p g a", a=A_RED)
                den = denp.tile([128, TG * INPUT], f32, name="den")
                rec = (None if (POOL_DIV or DIV_TT)
                       else denp.tile([128, TG * INPUT], f32, name="rec"))
                keep = A_RED
                if FOLD:
                    keep = A_RED - FOLD
                    nc.gpsimd.tensor_tensor(
                        out=g[:, :, 0:FOLD], in0=g[:, :, 0:FOLD],
                        in1=g[:, :, keep:A_RED], op=mybir.AluOpType.add,
                    )
                nc.vector.tensor_reduce(
                    out=den, in_=g[:, :, 0:keep],
                    axis=mybir.AxisListType.X, op=mybir.AluOpType.add,
                )
                if DROP_A1:
                    if ADD1_ON == "pool":
                        nc.gpsimd.tensor_scalar_add(out=den, in0=den,
                                                    scalar1=1.0)
                    else:
                        nc.vector.tensor_scalar_add(out=den, in0=den,
                                                    scalar1=1.0)
                xs = x_sb[:, b0:b0 + TG, :]
                ys = out_sb[:, b0:b0 + TG, :]
                if DIV_TT:
                    nc.vector.tensor_tensor(
                        out=ys, in0=xs,
                        in1=den.rearrange("p (t f) -> p t f", f=INPUT),
                        op=mybir.AluOpType.divide,
                    )
                elif POOL_DIV:
                    nc.gpsimd.tensor_tensor(
                        out=ys, in0=xs,
                        in1=den.rearrange("p (t f) -> p t f", f=INPUT),
                        op=mybir.AluOpType.divide,
                    )
                else:
                    nc.vector.reciprocal(out=rec, in_=den)
                    rec3 = rec.rearrange("p (t f) -> p t f", f=INPUT)
                    if MUL_ON == "pool":
                        nc.gpsimd.tensor_tensor(
                            out=ys, in0=xs, in1=rec3,
                            op=mybir.AluOpType.mult,
                        )
                    else:
                        nc.vector.tensor_tensor(
                            out=ys, in0=xs, in1=rec3,
                            op=mybir.AluOpType.mult,
                        )

            if off == DM - 1:
                m0 = iters[git * DM]
                og = ogs.pop(git)
                if ABLATE != "out":
                    nc.sync.dma_start(
                        out=y_r[m0 * NBLK:m0 * NBLK + DM * NBLK]
                        .rearrange("m p f -> p m f"),
                        in_=og,
                    )

    nc.compile()
    return nc


def _prep_weights(E_W, E_b, A_W, A_b):
    E_W = np.asarray(E_W, dtype=np.float32)
    E_b = np.asarray(E_b, dtype=np.float32)
    A_W = np.asarray(A_W, dtype=np.float32)
    A_b = np.asarray(A_b, dtype=np.float32)
    w1 = np.concatenate([E_W, np.zeros((INPUT, 1), np.float32)], axis=1)
    b1 = np.concatenate([E_b, np.float32([CONST_ROW_BIAS])]).reshape(-1, 1)
    _, NIA, _ = _dims()
    dW = A_W - A_W[:, :, 1:2]                        # [66, 50, 20]
    db = A_b - A_b[:, 1:2]                           # [66, 20]
    if DROP_A1:
        dW = np.delete(dW, 1, axis=2)                # [66, 50, 19]
        db = np.delete(db, 1, axis=1)                # [66, 19]
    w2 = np.concatenate(
        [dW.transpose(1, 0, 2).reshape(E_NODE, NIA),
         db.reshape(1, NIA)], axis=0,
    ).astype(np.float32)                             # [51, NIA]
    return np.ascontiguousarray(w1), np.ascontiguousarray(b1), \
        np.ascontiguousarray(w2)


def _run(x, E_W, E_b, A_W, A_b, trace=False):
    from concourse.bass_utils import run_bass_kernel_spmd

    x = np.ascontiguousarray(np.asarray(x, dtype=np.float32))
    n_rows_local = x.shape[0] // N_CORES
    key = ("nc", n_rows_local)
    if key not in _CACHE:
        _CACHE[key] = _build_bass(n_rows_local)
    nc = _CACHE[key]

    w1, b1, w2 = _prep_weights(E_W, E_b, A_W, A_b)
    in_maps = [
        {"x": x[i * n_rows_local:(i + 1) * n_rows_local],
         "W1": w1, "b1": b1, "W2": w2}
        for i in range(N_CORES)
    ]
    res = run_bass_kernel_spmd(nc, in_maps, list(range(N_CORES)), trace=trace)
    out = np.concatenate([res.results[i]["y"] for i in range(N_CORES)], axis=0)
    return out, res


def kernel(x, E_W, E_b, A_W, A_b):
    out, _ = _run(x, E_W, E_b, A_W, A_b, trace=False)
    return out
